# revision 4
# baseline (speedup 1.0000x reference)
"""Trainium2 Bass kernel for nn_Curvature (topk_masking).

Pipeline per NeuronCore (8 cores, 4 samples each, pure data parallel):
  1. Host pre-splits x into an exact fp16 (hi, lo) pair and pre-shuffles it
     into the conv pair-layout [128p = 2ch x 64rows, (hi/lo, 4gg, 8k, 64w)]
     so every DMA descriptor is a 4KB contiguous run (full ring rate).
  2. Depthwise 3x3 conv as 6 accumulating PE matmuls per 16-channel group
     against banded stationary matrices built from the weight (one per
     column shift dj; hi and lo streamed through the same stationary).
  3. |conv| row-sums on DVE (tensor_reduce with absolute value, two groups
     per op), baseline-subtracted for fp32 accuracy, then per-channel
     totals via PE transpose + DVE reduce -> per-sample channel scores.
  4. Top-k (k=256) as counting-rank: rank(c) = #{j: s_j > s_c} +
     #{j < c: s_j == s_c} (matches jax.lax.top_k tie-breaking). The
     all-channel score row is broadcast to 128 partitions with a PE
     transpose + tiny selector matmuls (no DRAM bounce); the inverse
     permutation is computed on DVE with is_equal counting against the
     rank broadcast (no scatter round trip).
  5. Gather the selected planes from a host-staged fp16 copy by rank via
     indirect DMA (8KB descriptors) and write the output with a SWDGE
     fp16->fp32 casting DMA. Gather+write live on the gpsimd queue so the
     sync/scalar HWDGE rings only ever stream conv tiles (no head-of-line
     blocking of the PE).
  Scores/topk for sample s-1 are emitted interleaved into sample s's conv
  stream so every engine stays busy; only the last sample's topk+gather is
  exposed as a short tail.
"""
import sys
import numpy as np

sys.path.insert(0, "/opt/trn_rl_repo")

import concourse.bacc as bacc
import concourse.bass as bass
import concourse.mybir as mybir
from concourse.masks import make_identity
from concourse.tile import TileContext
from concourse.bass_utils import run_bass_kernel_spmd

B, C, H, W = 32, 512, 64, 64
K = C // 2                 # 256 channels kept
NCORES = 8
SPC = B // NCORES          # samples per core = 4
HO, WO = H - 2, W - 2      # 62 x 62 valid conv output
NG = C // 16               # 32 groups of 16 channels (8 pairs)
PLANE = H * W
GPG = 4                    # groups per DMA load
NGT = NG // GPG            # 8 tile-loads per sample

_nc_cache = {}


def _build_nc(n_terms: int):
    """One SPMD program: SPC samples, full score+topk+gather pipeline."""
    if n_terms in _nc_cache:
        return _nc_cache[n_terms]
    nc = bacc.Bacc()
    dt = mybir.dt
    f32, f16, i32 = dt.float32, dt.float16, dt.int32
    Alu = mybir.AluOpType
    Ax = mybir.AxisListType

    xh = nc.declare_dram_parameter("xh", [SPC * C, PLANE], f16, isOutput=False)
    xhl = nc.declare_dram_parameter(
        "xhl", [SPC * NGT * 128, 2 * GPG * 8 * 64], f16, isOutput=False)
    bh = nc.declare_dram_parameter("bh", [128, 3 * 124], f16, isOutput=False)
    if n_terms == 3:
        bl = nc.declare_dram_parameter("bl", [128, 3 * 124], f16, isOutput=False)
    mb = nc.declare_dram_parameter("mb", [128, 1], f32, isOutput=False)
    ltm = nc.declare_dram_parameter("ltm", [128, 4 * 512], f32, isOutput=False)
    cvw = nc.declare_dram_parameter("cvw", [128, 512], f32, isOutput=False)
    eo = nc.declare_dram_parameter("eo", [4, 512], f32, isOutput=False)
    rpos = nc.declare_dram_parameter("rpos", [128, 2], f32, isOutput=False)
    out = nc.declare_dram_parameter("out", [SPC * K, PLANE], f32, isOutput=True)
    sdbg = nc.declare_dram_parameter("sdbg", [SPC * 4, 128], f32, isOutput=True)

    xhv = xhl[:].rearrange("(s G p) f -> s G p f", s=SPC, G=NGT, p=128)

    with TileContext(nc) as tc:
        with tc.tile_pool(name="cst", bufs=1) as cst, \
             tc.tile_pool(name="xtp", bufs=10) as xtp, \
             tc.tile_pool(name="rp", bufs=2) as rp, \
             tc.tile_pool(name="sp", bufs=2) as sp, \
             tc.tile_pool(name="gp", bufs=2) as gp, \
             tc.tile_pool(name="pcp", bufs=2, space="PSUM") as pcp, \
             tc.tile_pool(name="ptp", bufs=1, space="PSUM") as ptp, \
             tc.tile_pool(name="pbz", bufs=1, space="PSUM") as pbz:

            t_bh = cst.tile([128, 3 * 124], f16)
            nc.sync.dma_start(out=t_bh[:], in_=bh[:])
            # sample-0 tile-0 next, split in two tiles with exact deps
            # so the first conv matmuls only wait for the first 512KB.
            xhv00 = xhv[0, 0].rearrange(
                "p (hl gg k w) -> p hl gg k w", hl=2, gg=GPG, k=8)
            xt00a = xtp.tile([128, 2, 1, 8, 64], f16, tag="xt0a",
                             name="xt00a", bufs=1)
            nc.sync.dma_start(out=xt00a[:], in_=xhv00[:, :, 0:1, :, :])
            xt00b = xtp.tile([128, 2, GPG - 1, 8, 64], f16, tag="xt0b",
                             name="xt00b", bufs=1)
            nc.scalar.dma_start(out=xt00b[:], in_=xhv00[:, :, 1:GPG, :, :])
            if n_terms == 3:
                t_bl = cst.tile([128, 3 * 124], f16)
                nc.sync.dma_start(out=t_bl[:], in_=bl[:])
            t_mb = cst.tile([128, 1], f32)
            nc.sync.dma_start(out=t_mb[:], in_=mb[:])
            t_eo = cst.tile([4, 512], f32)
            nc.sync.dma_start(out=t_eo[:], in_=eo[:])
            t_rpos = cst.tile([128, 2], f32)
            nc.sync.dma_start(out=t_rpos[:], in_=rpos[:])
            # big topk tables go on the gpsimd ring (idle until topk(s0))
            t_ltm = cst.tile([128, 4 * 512], f32)
            t_cvw = cst.tile([128, 512], f32)
            ident = cst.tile([128, 128], f32)
            make_identity(nc, ident[:])
            ones = cst.tile([128, 512], f32)
            nc.vector.memset(ones[:], 1.0)

            def emit_group(s, G, R, xt=None, xt2=None):
                """load + conv matmuls + |.| row-sums for tile G of sample s."""
                if xt is None:
                    xt = xtp.tile([128, 2, GPG, 8, 64], f16, tag="xt")
                    ldeng = nc.sync if (G % 2 == 0) else nc.scalar
                    ldeng.dma_start(out=xt[:], in_=xhv[s, G])

                def rhs_of(hl, gg, lo, hi):
                    if xt2 is not None and gg > 0:
                        return xt2[:, hl, gg - 1, :, lo:hi]
                    return xt[:, hl, 0 if xt2 is not None else gg, :, lo:hi]
                for gp2 in range(GPG // 2):
                    pc = pcp.tile([124, 2, 512], f32, tag="pc")
                    for half in range(2):
                        gg = gp2 * 2 + half
                        mms = [(t_bh, 0), (t_bh, 1)]
                        if n_terms == 3:
                            mms.append((t_bl, 0))
                        nmm = 3 * len(mms)
                        im = 0
                        for dj in range(3):
                            for (tb, hl) in mms:
                                nc.tensor.matmul(
                                    pc[:, half, 0:496],
                                    lhsT=tb[:, dj * 124:(dj + 1) * 124],
                                    rhs=rhs_of(hl, gg, dj, dj + 62),
                                    start=(im == 0), stop=(im == nmm - 1))
                                im += 1
                    g0 = G * GPG + gp2 * 2
                    nc.vector.tensor_reduce(
                        out=R[:, g0 * 8:(g0 + 2) * 8].rearrange(
                            "p (b k) -> p b k", b=2),
                        in_=pc[:, :, 0:496].rearrange(
                            "p b (k w) -> p b k w", k=8),
                        axis=Ax.X, op=Alu.add, apply_absolute_value=True)

            def bcast512(src4):
                """[128,4] SBUF -> [128,512] PSUM row-broadcast.

                transpose to [4,128], copy to SBUF, then 4 selector matmuls
                (eo row q = e_q x ones) spread the four 128-chunks across
                the free dim on all 128 partitions.
                """
                tp4 = ptp.tile([4, 128], f32, tag="tp4")
                nc.tensor.transpose(tp4[:], src4[:, 0:4], ident[:128, :128])
                sl4 = sp.tile([4, 128], f32, tag="sl4")
                nc.vector.tensor_copy(sl4[:], tp4[:])
                bz = pbz.tile([128, 512], f32, tag="bz")
                for q in range(4):
                    nc.tensor.matmul(
                        bz[:, q * 128:(q + 1) * 128],
                        lhsT=t_eo[:, q * 128:(q + 1) * 128],
                        rhs=sl4[:, :],
                        start=True, stop=True)
                return sl4, bz

            def emit_scores_a(st):
                """R -> per-channel scores sc [128p, 4] for sample st."""
                R = st["R"]
                Rp = rp.tile([124, 256], f32, tag="Rp")
                nc.vector.tensor_scalar(
                    Rp[:], R[:], t_mb[:124, :1], None, op0=Alu.subtract)
                sc = sp.tile([128, 4], f32, tag="sc")
                for fc in range(2):
                    ptr = ptp.tile([128, 128], f32, tag="tp")
                    nc.tensor.transpose(
                        ptr[:, :124], Rp[:, fc * 128:(fc + 1) * 128],
                        ident[:124, :124])
                    nc.vector.tensor_reduce(
                        out=sc[:, fc * 2:fc * 2 + 2],
                        in_=ptr[:, :124].rearrange("p (par i) -> p par i", par=2),
                        axis=Ax.X, op=Alu.add)
                st["sc"] = sc

            def emit_scores_b(st):
                """broadcast scores to [128,512] psum + sdbg write."""
                s = st["s"]
                sl4, sbz = bcast512(st["sc"])
                nc.gpsimd.dma_start(out=sdbg[s * 4:(s + 1) * 4, :], in_=sl4[:])
                st["sbz"] = sbz

            def emit_ranks_a(st):
                """count strictly-greater scores for each channel."""
                sc, sbz = st["sc"], st["sbz"]
                cntg = sp.tile([128, 4], f32, tag="cntg")
                for q in range(4):
                    junk = sp.tile([128, 512], f32, tag="junk")
                    nc.vector.scalar_tensor_tensor(
                        out=junk[:], in0=sbz[:], scalar=sc[:, q:q + 1],
                        in1=ones[:], op0=Alu.is_gt, op1=Alu.mult,
                        accum_out=cntg[:, q:q + 1])
                st["cntg"] = cntg

            def emit_ranks_b(st):
                """tie-break counts + combine -> ranks [128,4] f32."""
                sc, sbz = st["sc"], st["sbz"]
                cnte = sp.tile([128, 4], f32, tag="cnte")
                for q in range(4):
                    junk = sp.tile([128, 512], f32, tag="junk")
                    nc.vector.scalar_tensor_tensor(
                        out=junk[:], in0=sbz[:], scalar=sc[:, q:q + 1],
                        in1=t_ltm[:, q * 512:(q + 1) * 512],
                        op0=Alu.is_equal, op1=Alu.mult,
                        accum_out=cnte[:, q:q + 1])
                ranks = sp.tile([128, 4], f32, tag="ranks")
                with nc.allow_low_precision(reason="exact small-int add"):
                    nc.vector.tensor_tensor(
                        out=ranks[:], in0=st["cntg"][:], in1=cnte[:],
                        op=Alu.add)
                st["ranks"] = ranks

            def emit_invert_a(st):
                """broadcast ranks to [128,512] psum."""
                _, rb = bcast512(st["ranks"])
                st["rb"] = rb

            def emit_invert_b(st):
                """inverse permutation on DVE: eraw[p,rc] = channel row with
                rank rc*128+p (as xh row id, sample base added)."""
                s, rb = st["s"], st["rb"]
                invf = sp.tile([128, 2], f32, tag="invf")
                for rc in range(2):
                    junk = sp.tile([128, 512], f32, tag="junk")
                    nc.vector.scalar_tensor_tensor(
                        out=junk[:], in0=rb[:], scalar=t_rpos[:, rc:rc + 1],
                        in1=t_cvw[:], op0=Alu.is_equal, op1=Alu.mult,
                        accum_out=invf[:, rc:rc + 1])
                erf = sp.tile([128, 2], f32, tag="erf")
                nc.vector.tensor_scalar(
                    erf[:], invf[:], float(s * C), None, op0=Alu.add)
                eraw = sp.tile([128, 2], i32, tag="eraw")
                nc.vector.tensor_copy(eraw[:], erf[:])
                st["eraw"] = eraw

            def emit_gather(st, rc):
                """gather one half of the selected fp16 planes into SBUF."""
                if rc == 0:
                    st["gt"] = gp.tile([128, 2, PLANE], f16, tag="gt",
                                       name="gt")
                nc.gpsimd.indirect_dma_start(
                    out=st["gt"][:, rc, :], out_offset=None, in_=xh[:],
                    in_offset=bass.IndirectOffsetOnAxis(
                        ap=st["eraw"][:, rc:rc + 1], axis=0))

            def emit_write(st):
                """write the gathered planes out, casting fp16->fp32."""
                s = st["s"]
                nc.gpsimd.dma_start(
                    out=out[s * K:(s + 1) * K, :].rearrange(
                        "(j p) f -> p j f", j=2),
                    in_=st["gt"][:])

            # software pipeline: topk(s-1) interleaved into conv stream (s)
            HOOKS = {0: emit_scores_a, 1: emit_scores_b,
                     2: emit_ranks_a, 3: emit_ranks_b,
                     4: emit_invert_a, 5: emit_invert_b,
                     6: lambda st: (emit_gather(st, 0), emit_gather(st, 1)),
                     7: emit_write}
            prev = None
            for s in range(SPC):
                st = {"s": s}
                st["R"] = rp.tile([124, 256], f32, tag="R", name="R")
                for G in range(NGT):
                    if prev is not None and G in HOOKS:
                        HOOKS[G](prev)
                    if s == 0 and G == 4:
                        # topk tables are first needed by topk(s0)
                        nc.gpsimd.dma_start(out=t_ltm[:], in_=ltm[:])
                        nc.gpsimd.dma_start(out=t_cvw[:], in_=cvw[:])
                    if s == 0 and G == 0:
                        emit_group(s, G, st["R"], xt=xt00a, xt2=xt00b)
                    else:
                        emit_group(s, G, st["R"])
                prev = st
            # drain the last sample
            emit_scores_a(prev)
            emit_scores_b(prev)
            emit_ranks_a(prev)
            emit_ranks_b(prev)
            emit_invert_a(prev)
            emit_invert_b(prev)
            emit_gather(prev, 0)
            emit_gather(prev, 1)
            emit_write(prev)
    nc.compile()
    _nc_cache[n_terms] = nc
    return nc


def _host_inputs(x: np.ndarray, weight: np.ndarray):
    w = weight.reshape(3, 3).astype(np.float32)
    wh = w.astype(np.float16)
    exact16 = bool(np.all(wh.astype(np.float32) == w))
    n_terms = 2 if exact16 else 3

    def banded(wcol):
        Bm = np.zeros((128, 3 * 124), dtype=np.float64)
        for dj in range(3):
            for half in range(2):
                for i in range(HO):
                    for t in range(3):
                        Bm[half * 64 + i + t, dj * 124 + half * 62 + i] = wcol[t, dj]
        return Bm

    Bfull = banded(w.astype(np.float64))
    bh_np = Bfull.astype(np.float16)
    bl_np = (Bfull - bh_np.astype(np.float64)).astype(np.float16)

    # baseline m: mean |conv| row-sum from one plane (ordering-neutral shift)
    p0 = x[0, 0].astype(np.float32)
    c0 = np.zeros((HO, WO), dtype=np.float32)
    for di in range(3):
        for dj in range(3):
            c0 += w[di, dj] * p0[di:di + HO, dj:dj + WO]
    m = np.float32(round(float(np.abs(c0).sum(axis=1).mean())))
    mb_np = np.full((128, 1), m, dtype=np.float32)

    # free position j = q*128 + f <-> channel chanmap[j]
    p = np.arange(128)
    j = np.arange(512)
    q_of_j, f_of_j = j // 128, j % 128
    chanmap = 256 * (q_of_j // 2) + 2 * f_of_j + (q_of_j % 2)
    # ltm[p, q*512+j] = 1 if chan(j) < chan(p, q)  (tie-break mask)
    ltm_np = np.zeros((128, 4 * 512), dtype=np.float32)
    for q in range(4):
        chan_pq = 256 * (q // 2) + 2 * p + (q % 2)
        ltm_np[:, q * 512:(q + 1) * 512] = (
            chanmap[None, :] < chan_pq[:, None]).astype(np.float32)
    cvw_np = np.broadcast_to(
        chanmap.astype(np.float32)[None, :], (128, 512)).copy()
    eo_np = np.zeros((4, 512), dtype=np.float32)
    for q in range(4):
        eo_np[q, q * 128:(q + 1) * 128] = 1.0
    rpos_np = np.empty((128, 2), dtype=np.float32)
    rpos_np[:, 0] = p
    rpos_np[:, 1] = 128 + p
    shared = dict(bh=bh_np, mb=mb_np, ltm=ltm_np, cvw=cvw_np,
                  eo=eo_np, rpos=rpos_np)
    if n_terms == 3:
        shared["bl"] = bl_np
    return n_terms, shared


def _split_pair_layout(xc: np.ndarray) -> np.ndarray:
    """fp16 (hi, lo) split of one core's x in the conv pair-layout.

    xc: [SPC*C, PLANE] fp32 ->
    [SPC*NGT*128, 2*GPG*8*64] fp16 where row (s, G, par*64+h) holds
    [hl, gg, k, w] contiguously (4KB per DMA descriptor).
    """
    xh = xc.astype(np.float16)
    xl = (xc - xh.astype(np.float32)).astype(np.float16)
    # channel c = ((G*GPG + gg)*8 + k)*2 + par
    # [2hl, s, G, gg, k, par, h, w]
    arr = np.stack([xh, xl]).reshape(2, SPC, NGT, GPG, 8, 2, H, W)
    # -> [s, G, par, h, hl, gg, k, w]
    arr = arr.transpose(1, 2, 5, 6, 0, 3, 4, 7)
    return np.ascontiguousarray(arr).reshape(SPC * NGT * 128, 2 * GPG * 8 * 64)


def run(x, weight, trace=False):
    x = np.ascontiguousarray(np.asarray(x, dtype=np.float32))
    weight = np.asarray(weight, dtype=np.float32)
    assert x.shape == (B, C, H, W), x.shape
    n_terms, shared = _host_inputs(x, weight)
    nc = _build_nc(n_terms)
    in_maps = []
    for d in range(NCORES):
        im = dict(shared)
        xc = x[d * SPC:(d + 1) * SPC].reshape(SPC * C, PLANE)
        im["xh"] = xc.astype(np.float16)
        im["xhl"] = _split_pair_layout(xc)
        in_maps.append(im)
    res = run_bass_kernel_spmd(nc, in_maps, core_ids=list(range(NCORES)),
                               trace=trace)
    outs = [res.results[d]["out"].reshape(SPC, K, H, W) for d in range(NCORES)]
    return np.concatenate(outs, axis=0), res


def kernel(x, weight):
    out, _ = run(x, weight, trace=False)
    return out


# revision 9
# speedup vs baseline: 1.0394x; 1.0394x over previous
"""Trainium2 Bass kernel for nn_Curvature (topk_masking).

Pipeline per NeuronCore (8 cores, 4 samples each, pure data parallel):
  1. Host pre-splits x into an exact fp16 (hi, lo) pair and pre-shuffles it
     into the conv pair-layout [128p = 2ch x 64rows, (hi/lo, 4gg, 8k, 64w)]
     so every DMA descriptor is a 4KB contiguous run (full ring rate).
  2. Depthwise 3x3 conv as 6 accumulating PE matmuls per 16-channel group
     against banded stationary matrices built from the weight (one per
     column shift dj; hi and lo streamed through the same stationary).
  3. |conv| row-sums on DVE (tensor_reduce with absolute value, two groups
     per op), baseline-subtracted for fp32 accuracy, then per-channel
     totals via PE transpose + DVE reduce -> per-sample channel scores.
  4. Top-k (k=256) as counting-rank: rank(c) = #{j: s_j > s_c} +
     #{j < c: s_j == s_c} (matches jax.lax.top_k tie-breaking). The
     all-channel score row is broadcast to [128, 512] PSUM with a PE
     transpose + fp16 hi/lo selector matmuls; the comparison scalar is
     recomputed as fp32(hi)+fp32(lo) on DVE so it is bit-identical to the
     PSUM value (counting stays exact). Greater-counts run on GpSimd in
     parallel with tie-break counts on DVE. The inverse permutation is
     an is_equal reduction against the (exact-integer) fp16-broadcast
     ranks -- no DRAM scatter round trip.
  5. Gather the selected planes from a host-staged fp16 copy by rank via
     indirect DMA (8KB descriptors) and write the output with SWDGE
     fp16->fp32 casting DMAs. Gather+write live on the gpsimd queue so
     the sync/scalar HWDGE rings only ever stream conv tiles.
  Scores/topk for sample s-1 are emitted interleaved into sample s's conv
  stream; only the last sample's topk+gather is exposed as a short tail.
"""
import sys
import numpy as np

sys.path.insert(0, "/opt/trn_rl_repo")

import concourse.bacc as bacc
import concourse.bass as bass
import concourse.mybir as mybir
from concourse.masks import make_identity
from concourse.tile import TileContext
from concourse.bass_utils import run_bass_kernel_spmd

B, C, H, W = 32, 512, 64, 64
K = C // 2                 # 256 channels kept
NCORES = 8
SPC = B // NCORES          # samples per core = 4
HO, WO = H - 2, W - 2      # 62 x 62 valid conv output
NG = C // 16               # 32 groups of 16 channels (8 pairs)
PLANE = H * W
GPG = 4                    # groups per DMA load
NGT = NG // GPG            # 8 tile-loads per sample

_nc_cache = {}


def _build_nc(n_terms: int):
    """One SPMD program: SPC samples, full score+topk+gather pipeline."""
    if n_terms in _nc_cache:
        return _nc_cache[n_terms]
    nc = bacc.Bacc()
    dt = mybir.dt
    f32, f16, i32 = dt.float32, dt.float16, dt.int32
    Alu = mybir.AluOpType
    Ax = mybir.AxisListType

    xh = nc.declare_dram_parameter("xh", [SPC * C, PLANE], f16, isOutput=False)
    xhl = nc.declare_dram_parameter(
        "xhl", [SPC * NGT * 128, 2 * GPG * 8 * 64], f16, isOutput=False)
    bh = nc.declare_dram_parameter("bh", [128, 3 * 124], f16, isOutput=False)
    if n_terms == 3:
        bl = nc.declare_dram_parameter("bl", [128, 3 * 124], f16, isOutput=False)
    mb1 = nc.declare_dram_parameter("mb1", [1, 128], f32, isOutput=False)
    ltm = nc.declare_dram_parameter("ltm", [128, 4 * 512], f32, isOutput=False)
    cvw = nc.declare_dram_parameter("cvw", [128, 512], f32, isOutput=False)
    eo = nc.declare_dram_parameter("eo", [4, 512], f16, isOutput=False)
    out = nc.declare_dram_parameter("out", [SPC * K, PLANE], f32, isOutput=True)
    sdbg = nc.declare_dram_parameter("sdbg", [SPC * 4, 128], f32, isOutput=True)

    xhv = xhl[:].rearrange("(s G p) f -> s G p f", s=SPC, G=NGT, p=128)

    with TileContext(nc) as tc:
        with tc.tile_pool(name="cst", bufs=1) as cst, \
             tc.tile_pool(name="xtp", bufs=10) as xtp, \
             tc.tile_pool(name="rp", bufs=2) as rp, \
             tc.tile_pool(name="sp", bufs=2) as sp, \
             tc.tile_pool(name="gp", bufs=2) as gp, \
             tc.tile_pool(name="pcp", bufs=2, space="PSUM") as pcp, \
             tc.tile_pool(name="ptp", bufs=1, space="PSUM") as ptp, \
             tc.tile_pool(name="pbz", bufs=1, space="PSUM") as pbz:

            # critical path first: bh then the first conv tile on the sync
            # ring; everything tiny or big-but-late goes on gpsimd (SWDGE)
            # so the HWDGE rings never see small-descriptor storms.
            t_bh = cst.tile([128, 3 * 124], f16)
            nc.sync.dma_start(out=t_bh[:], in_=bh[:])
            xhv00 = xhv[0, 0].rearrange(
                "p (hl gg k w) -> p hl gg k w", hl=2, gg=GPG, k=8)
            xt00a = xtp.tile([128, 2, 1, 8, 64], f16, tag="xt0a",
                             name="xt00a", bufs=1)
            nc.sync.dma_start(out=xt00a[:], in_=xhv00[:, :, 0:1, :, :])
            xt00b = xtp.tile([128, 2, GPG - 1, 8, 64], f16, tag="xt0b",
                             name="xt00b", bufs=1)
            nc.scalar.dma_start(out=xt00b[:], in_=xhv00[:, :, 1:GPG, :, :])
            if n_terms == 3:
                t_bl = cst.tile([128, 3 * 124], f16)
                nc.sync.dma_start(out=t_bl[:], in_=bl[:])
            t_eo = cst.tile([4, 512], f16)
            nc.gpsimd.dma_start(out=t_eo[:], in_=eo[:])
            t_mb1 = cst.tile([1, 128], f32)
            nc.gpsimd.dma_start(out=t_mb1[:], in_=mb1[:])
            t_ltm = cst.tile([128, 4 * 512], f32)
            nc.gpsimd.dma_start(out=t_ltm[:], in_=ltm[:])
            t_cvw = cst.tile([128, 512], f32)
            nc.gpsimd.dma_start(out=t_cvw[:], in_=cvw[:])
            rpi = cst.tile([128, 2], i32)
            nc.gpsimd.iota(rpi[:], pattern=[[128, 2]], base=0,
                           channel_multiplier=1)
            t_rpos = cst.tile([128, 2], f32)
            nc.vector.tensor_copy(t_rpos[:], rpi[:])
            ident = cst.tile([128, 128], f32)
            make_identity(nc, ident[:])
            ones = cst.tile([128, 512], f32)
            nc.vector.memset(ones[:], 1.0)
            t_mb = cst.tile([128, 1], f32)

            def emit_mb():
                """[1,128] -> [128,1] per-partition baseline via PE."""
                pm = ptp.tile([128, 128], f32, tag="tp")
                nc.tensor.transpose(pm[:, 0:1], t_mb1[:, :], ident[:1, :1])
                nc.vector.tensor_copy(t_mb[:], pm[:, 0:1])

            def emit_group(s, G, R, xt=None, xt2=None):
                """load + conv matmuls + |.| row-sums for tile G of sample s."""
                if xt is None:
                    xt = xtp.tile([128, 2, GPG, 8, 64], f16, tag="xt")
                    ldeng = nc.sync if (G % 2 == 0) else nc.scalar
                    ldeng.dma_start(out=xt[:], in_=xhv[s, G])

                def rhs_of(hl, gg, lo, hi):
                    if xt2 is not None and gg > 0:
                        return xt2[:, hl, gg - 1, :, lo:hi]
                    return xt[:, hl, 0 if xt2 is not None else gg, :, lo:hi]
                for gp2 in range(GPG // 2):
                    pc = pcp.tile([124, 2, 512], f32, tag="pc")
                    for half in range(2):
                        gg = gp2 * 2 + half
                        mms = [(t_bh, 0), (t_bh, 1)]
                        if n_terms == 3:
                            mms.append((t_bl, 0))
                        nmm = 3 * len(mms)
                        im = 0
                        for dj in range(3):
                            for (tb, hl) in mms:
                                nc.tensor.matmul(
                                    pc[:, half, 0:496],
                                    lhsT=tb[:, dj * 124:(dj + 1) * 124],
                                    rhs=rhs_of(hl, gg, dj, dj + 62),
                                    start=(im == 0), stop=(im == nmm - 1))
                                im += 1
                    g0 = G * GPG + gp2 * 2
                    nc.vector.tensor_reduce(
                        out=R[:, g0 * 8:(g0 + 2) * 8].rearrange(
                            "p (b k) -> p b k", b=2),
                        in_=pc[:, :, 0:496].rearrange(
                            "p b (k w) -> p b k w", k=8),
                        axis=Ax.X, op=Alu.add, apply_absolute_value=True)

            def emit_scores_a(st):
                """R -> per-channel scores sc [128p, 4] for sample st."""
                R = st["R"]
                Rp = rp.tile([124, 256], f32, tag="Rp")
                nc.vector.tensor_scalar(
                    Rp[:], R[:], t_mb[:124, :1], None, op0=Alu.subtract)
                sc = sp.tile([128, 4], f32, tag="sc")
                for fc in range(2):
                    ptr = ptp.tile([128, 128], f32, tag="tp")
                    nc.tensor.transpose(
                        ptr[:, :124], Rp[:, fc * 128:(fc + 1) * 128],
                        ident[:124, :124])
                    nc.vector.tensor_reduce(
                        out=sc[:, fc * 2:fc * 2 + 2],
                        in_=ptr[:, :124].rearrange("p (par i) -> p par i", par=2),
                        axis=Ax.X, op=Alu.add)
                st["sc"] = sc

            def emit_split(st):
                """exact fp16 hi/lo split of sc + the bit-consistent scalar
                sstar = fp32(hi) + fp32(lo) (same arithmetic as PSUM)."""
                sc = st["sc"]
                sch16 = sp.tile([128, 4], f16, tag="sch16")
                nc.vector.tensor_copy(sch16[:], sc[:])
                schf = sp.tile([128, 4], f32, tag="schf")
                nc.vector.tensor_copy(schf[:], sch16[:])
                dif = sp.tile([128, 4], f32, tag="dif")
                with nc.allow_low_precision(reason="exact residual"):
                    nc.vector.tensor_tensor(
                        out=dif[:], in0=sc[:], in1=schf[:], op=Alu.subtract)
                scl16 = sp.tile([128, 4], f16, tag="scl16")
                nc.vector.tensor_copy(scl16[:], dif[:])
                sclf = sp.tile([128, 4], f32, tag="sclf")
                nc.vector.tensor_copy(sclf[:], scl16[:])
                sstar = sp.tile([128, 4], f32, tag="sstar")
                with nc.allow_low_precision(reason="matches psum add"):
                    nc.vector.tensor_tensor(
                        out=sstar[:], in0=schf[:], in1=sclf[:], op=Alu.add)
                st["schf"], st["sclf"], st["sstar"] = schf, sclf, sstar

            def tr_row16(src, tag):
                """[128,4] f32 -> [4,128] f16 row tile via PE transpose."""
                tp4 = ptp.tile([4, 128], f32, tag="tp4")
                nc.tensor.transpose(tp4[:], src[:, 0:4], ident[:128, :128])
                r16 = sp.tile([4, 128], f16, tag=tag, name="r16_" + tag)
                nc.vector.tensor_copy(r16[:], tp4[:])
                return r16

            def emit_bcast_scores(st):
                """fp16 hi/lo selector matmuls -> sbz [128,512] psum."""
                s = st["s"]
                slh = tr_row16(st["schf"], "slh")
                sll = tr_row16(st["sclf"], "sll")
                sbz = pbz.tile([128, 512], f32, tag="bz", name="sbz")
                for q in range(4):
                    nc.tensor.matmul(
                        sbz[:, q * 128:(q + 1) * 128],
                        lhsT=t_eo[:, q * 128:(q + 1) * 128],
                        rhs=slh[:, :], start=True, stop=False)
                    nc.tensor.matmul(
                        sbz[:, q * 128:(q + 1) * 128],
                        lhsT=t_eo[:, q * 128:(q + 1) * 128],
                        rhs=sll[:, :], start=False, stop=True)
                nc.gpsimd.dma_start(out=sdbg[s * 4:(s + 1) * 4, :],
                                    in_=slh[:])
                st["sbz"] = sbz

            def emit_ranks_g(st):
                """greater-counts (DVE; gpsimd lacks the TensorScalarPtr op)."""
                sstar, sbz = st["sstar"], st["sbz"]
                cntg = sp.tile([128, 4], f32, tag="cntg")
                for q in range(4):
                    junk = sp.tile([128, 512], f32, tag="junkg")
                    nc.vector.scalar_tensor_tensor(
                        out=junk[:], in0=sbz[:], scalar=sstar[:, q:q + 1],
                        in1=ones[:], op0=Alu.is_gt, op1=Alu.mult,
                        accum_out=cntg[:, q:q + 1])
                st["cntg"] = cntg

            def emit_ranks_e(st):
                """tie-break counts on DVE + combine -> ranks [128,4]."""
                sstar, sbz = st["sstar"], st["sbz"]
                cnte = sp.tile([128, 4], f32, tag="cnte")
                for q in range(4):
                    junk = sp.tile([128, 512], f32, tag="junk")
                    nc.vector.scalar_tensor_tensor(
                        out=junk[:], in0=sbz[:], scalar=sstar[:, q:q + 1],
                        in1=t_ltm[:, q * 512:(q + 1) * 512],
                        op0=Alu.is_equal, op1=Alu.mult,
                        accum_out=cnte[:, q:q + 1])
                ranks = sp.tile([128, 4], f32, tag="ranks")
                with nc.allow_low_precision(reason="exact small-int add"):
                    nc.vector.tensor_tensor(
                        out=ranks[:], in0=st["cntg"][:], in1=cnte[:],
                        op=Alu.add)
                st["ranks"] = ranks

            def emit_bcast_ranks(st):
                """ranks are exact ints <=511: single fp16 selector pass."""
                rkh = tr_row16(st["ranks"], "rkh")
                rb = pbz.tile([128, 512], f32, tag="bz", name="rb")
                for q in range(4):
                    nc.tensor.matmul(
                        rb[:, q * 128:(q + 1) * 128],
                        lhsT=t_eo[:, q * 128:(q + 1) * 128],
                        rhs=rkh[:, :], start=True, stop=True)
                st["rb"] = rb

            def emit_invert(st):
                """inverse permutation: eraw[p,rc] = xh row with rank rc*128+p."""
                s, rb = st["s"], st["rb"]
                invf = sp.tile([128, 2], f32, tag="invf")
                for rc in range(2):
                    junk = sp.tile([128, 512], f32, tag="junk")
                    nc.vector.scalar_tensor_tensor(
                        out=junk[:], in0=rb[:], scalar=t_rpos[:, rc:rc + 1],
                        in1=t_cvw[:], op0=Alu.is_equal, op1=Alu.mult,
                        accum_out=invf[:, rc:rc + 1])
                erf = sp.tile([128, 2], f32, tag="erf")
                nc.vector.tensor_scalar(
                    erf[:], invf[:], float(s * C), None, op0=Alu.add)
                eraw = sp.tile([128, 2], i32, tag="eraw")
                nc.vector.tensor_copy(eraw[:], erf[:])
                st["eraw"] = eraw

            def emit_gather(st, rc):
                """gather one half of the selected fp16 planes into SBUF."""
                if rc == 0:
                    st["gt"] = gp.tile([128, 2, PLANE], f16, tag="gt",
                                       name="gt")
                nc.gpsimd.indirect_dma_start(
                    out=st["gt"][:, rc, :], out_offset=None, in_=xh[:],
                    in_offset=bass.IndirectOffsetOnAxis(
                        ap=st["eraw"][:, rc:rc + 1], axis=0))

            def emit_write(st, rc):
                """write one half of the output, casting fp16->fp32."""
                s = st["s"]
                nc.gpsimd.dma_start(
                    out=out[s * K + rc * 128:s * K + (rc + 1) * 128, :],
                    in_=st["gt"][:, rc, :])

            # software pipeline: topk(s-1) interleaved into conv stream (s)
            HOOKS = {0: emit_scores_a, 1: emit_split,
                     2: emit_bcast_scores, 3: emit_ranks_g,
                     4: emit_ranks_e, 5: emit_bcast_ranks,
                     6: lambda st: (emit_invert(st), emit_gather(st, 0),
                                    emit_gather(st, 1)),
                     7: lambda st: (emit_write(st, 0), emit_write(st, 1))}
            prev = None
            for s in range(SPC):
                st = {"s": s}
                st["R"] = rp.tile([124, 256], f32, tag="R", name="R")
                for G in range(NGT):
                    if prev is not None and G in HOOKS:
                        HOOKS[G](prev)
                    if s == 0 and G == 5:
                        emit_mb()
                    if s == 0 and G == 0:
                        emit_group(s, G, st["R"], xt=xt00a, xt2=xt00b)
                    else:
                        emit_group(s, G, st["R"])
                prev = st
            # drain the last sample
            emit_scores_a(prev)
            emit_split(prev)
            emit_bcast_scores(prev)
            emit_ranks_g(prev)
            emit_ranks_e(prev)
            emit_bcast_ranks(prev)
            emit_invert(prev)
            emit_gather(prev, 0)
            emit_gather(prev, 1)
            emit_write(prev, 0)
            emit_write(prev, 1)
    nc.compile()
    _nc_cache[n_terms] = nc
    return nc


def _host_inputs(x: np.ndarray, weight: np.ndarray):
    w = weight.reshape(3, 3).astype(np.float32)
    wh = w.astype(np.float16)
    exact16 = bool(np.all(wh.astype(np.float32) == w))
    n_terms = 2 if exact16 else 3

    def banded(wcol):
        Bm = np.zeros((128, 3 * 124), dtype=np.float64)
        for dj in range(3):
            for half in range(2):
                for i in range(HO):
                    for t in range(3):
                        Bm[half * 64 + i + t, dj * 124 + half * 62 + i] = wcol[t, dj]
        return Bm

    Bfull = banded(w.astype(np.float64))
    bh_np = Bfull.astype(np.float16)
    bl_np = (Bfull - bh_np.astype(np.float64)).astype(np.float16)

    # baseline m: mean |conv| row-sum from one plane (ordering-neutral shift)
    p0 = x[0, 0].astype(np.float32)
    c0 = np.zeros((HO, WO), dtype=np.float32)
    for di in range(3):
        for dj in range(3):
            c0 += w[di, dj] * p0[di:di + HO, dj:dj + WO]
    m = np.float32(round(float(np.abs(c0).sum(axis=1).mean())))
    mb1_np = np.full((1, 128), m, dtype=np.float32)

    # free position j = q*128 + f <-> channel chanmap[j]
    p = np.arange(128)
    j = np.arange(512)
    q_of_j, f_of_j = j // 128, j % 128
    chanmap = 256 * (q_of_j // 2) + 2 * f_of_j + (q_of_j % 2)
    # ltm[p, q*512+j] = 1 if chan(j) < chan(p, q)  (tie-break mask)
    ltm_np = np.zeros((128, 4 * 512), dtype=np.float32)
    for q in range(4):
        chan_pq = 256 * (q // 2) + 2 * p + (q % 2)
        ltm_np[:, q * 512:(q + 1) * 512] = (
            chanmap[None, :] < chan_pq[:, None]).astype(np.float32)
    cvw_np = np.broadcast_to(
        chanmap.astype(np.float32)[None, :], (128, 512)).copy()
    eo_np = np.zeros((4, 512), dtype=np.float16)
    for q in range(4):
        eo_np[q, q * 128:(q + 1) * 128] = 1.0
    shared = dict(bh=bh_np, mb1=mb1_np, ltm=ltm_np, cvw=cvw_np, eo=eo_np)
    if n_terms == 3:
        shared["bl"] = bl_np
    return n_terms, shared


def _split_pair_layout(xc: np.ndarray) -> np.ndarray:
    """fp16 (hi, lo) split of one core's x in the conv pair-layout.

    xc: [SPC*C, PLANE] fp32 ->
    [SPC*NGT*128, 2*GPG*8*64] fp16 where row (s, G, par*64+h) holds
    [hl, gg, k, w] contiguously (4KB per DMA descriptor).
    """
    xh = xc.astype(np.float16)
    xl = (xc - xh.astype(np.float32)).astype(np.float16)
    # channel c = ((G*GPG + gg)*8 + k)*2 + par
    # [2hl, s, G, gg, k, par, h, w]
    arr = np.stack([xh, xl]).reshape(2, SPC, NGT, GPG, 8, 2, H, W)
    # -> [s, G, par, h, hl, gg, k, w]
    arr = arr.transpose(1, 2, 5, 6, 0, 3, 4, 7)
    return np.ascontiguousarray(arr).reshape(SPC * NGT * 128, 2 * GPG * 8 * 64)


def run(x, weight, trace=False):
    x = np.ascontiguousarray(np.asarray(x, dtype=np.float32))
    weight = np.asarray(weight, dtype=np.float32)
    assert x.shape == (B, C, H, W), x.shape
    n_terms, shared = _host_inputs(x, weight)
    nc = _build_nc(n_terms)
    in_maps = []
    for d in range(NCORES):
        im = dict(shared)
        xc = x[d * SPC:(d + 1) * SPC].reshape(SPC * C, PLANE)
        im["xh"] = xc.astype(np.float16)
        im["xhl"] = _split_pair_layout(xc)
        in_maps.append(im)
    res = run_bass_kernel_spmd(nc, in_maps, core_ids=list(range(NCORES)),
                               trace=trace)
    outs = [res.results[d]["out"].reshape(SPC, K, H, W) for d in range(NCORES)]
    return np.concatenate(outs, axis=0), res


def kernel(x, weight):
    out, _ = run(x, weight, trace=False)
    return out


# revision 14
# speedup vs baseline: 1.0654x; 1.0250x over previous
"""Trainium2 Bass kernel for nn_Curvature (topk_masking).

Pipeline per NeuronCore (8 cores, 4 samples each, pure data parallel):
  1. Host pre-splits x into an exact fp16 (hi, lo) pair and pre-shuffles it
     into the conv pair-layout [128p = 2ch x 64rows, (hi/lo, 4gg, 8k, 64w)]
     so every DMA descriptor is a 4KB contiguous run (full ring rate).
  2. Depthwise 3x3 conv as 6 accumulating PE matmuls per 16-channel group
     against banded stationary matrices built from the weight (one per
     column shift dj; hi and lo streamed through the same stationary).
  3. |conv| row-sums on DVE (tensor_reduce with absolute value, two groups
     per op), baseline-subtracted for fp32 accuracy, then per-channel
     totals via PE transpose + DVE reduce -> per-sample channel scores.
  4. Top-k (k=256) as counting-rank: rank(c) = #{j: s_j > s_c} +
     #{j < c: s_j == s_c} (matches jax.lax.top_k tie-breaking). The
     all-channel score row is broadcast to [128, 512] PSUM with a PE
     transpose + fp16 hi/lo selector matmuls; the comparison scalar is
     recomputed as fp32(hi)+fp32(lo) on DVE so it is bit-identical to the
     PSUM value (counting stays exact). Greater-counts run on GpSimd in
     parallel with tie-break counts on DVE. The inverse permutation is
     an is_equal reduction against the (exact-integer) fp16-broadcast
     ranks -- no DRAM scatter round trip.
  5. Gather the selected planes from a host-staged fp16 copy by rank via
     indirect DMA (8KB descriptors) and write the output with SWDGE
     fp16->fp32 casting DMAs. Gather+write live on the gpsimd queue so
     the sync/scalar HWDGE rings only ever stream conv tiles.
  Scores/topk for sample s-1 are emitted interleaved into sample s's conv
  stream; only the last sample's topk+gather is exposed as a short tail.
"""
import sys
import numpy as np

sys.path.insert(0, "/opt/trn_rl_repo")

import concourse.bacc as bacc
import concourse.bass as bass
import concourse.mybir as mybir
from concourse.masks import make_identity
from concourse.tile import TileContext
from concourse.bass_utils import run_bass_kernel_spmd

B, C, H, W = 32, 512, 64, 64
K = C // 2                 # 256 channels kept
NCORES = 8
SPC = B // NCORES          # samples per core = 4
HO, WO = H - 2, W - 2      # 62 x 62 valid conv output
NG = C // 16               # 32 groups of 16 channels (8 pairs)
PLANE = H * W
GPG = 4                    # groups per DMA load
NGT = NG // GPG            # 8 tile-loads per sample

_nc_cache = {}


def _build_nc(n_terms: int):
    """One SPMD program: SPC samples, full score+topk+gather pipeline."""
    if n_terms in _nc_cache:
        return _nc_cache[n_terms]
    nc = bacc.Bacc()
    dt = mybir.dt
    f32, f16, i32 = dt.float32, dt.float16, dt.int32
    Alu = mybir.AluOpType
    Ax = mybir.AxisListType

    xh = nc.declare_dram_parameter("xh", [SPC * C, PLANE], f16, isOutput=False)
    xhl = nc.declare_dram_parameter(
        "xhl", [SPC * NGT * 128, 2 * GPG * 8 * 64], f16, isOutput=False)
    bh = nc.declare_dram_parameter("bh", [128, 3 * 124], f16, isOutput=False)
    if n_terms == 3:
        bl = nc.declare_dram_parameter("bl", [128, 3 * 124], f16, isOutput=False)
    mb1 = nc.declare_dram_parameter("mb1", [1, 128], f32, isOutput=False)
    ltm = nc.declare_dram_parameter("ltm", [128, 4 * 512], f32, isOutput=False)
    cvw = nc.declare_dram_parameter("cvw", [128, 512], f32, isOutput=False)
    eo = nc.declare_dram_parameter("eo", [4, 512], f16, isOutput=False)
    out = nc.declare_dram_parameter("out", [SPC * K, PLANE], f32, isOutput=True)
    sdbg = nc.declare_dram_parameter("sdbg", [SPC * 8, 128], f32, isOutput=True)

    xhv = xhl[:].rearrange("(s G p) f -> s G p f", s=SPC, G=NGT, p=128)

    with TileContext(nc) as tc:
        with tc.tile_pool(name="cst", bufs=1) as cst, \
             tc.tile_pool(name="xtp", bufs=10) as xtp, \
             tc.tile_pool(name="rp", bufs=2) as rp, \
             tc.tile_pool(name="sp", bufs=2) as sp, \
             tc.tile_pool(name="gp", bufs=2) as gp, \
             tc.tile_pool(name="pcp", bufs=2, space="PSUM") as pcp, \
             tc.tile_pool(name="ptp", bufs=1, space="PSUM") as ptp, \
             tc.tile_pool(name="pbz", bufs=1, space="PSUM") as pbz:

            # critical path first: bh then the first conv tile on the sync
            # ring; everything tiny or big-but-late goes on gpsimd (SWDGE)
            # so the HWDGE rings never see small-descriptor storms.
            t_bh = cst.tile([128, 3 * 124], f16)
            nc.sync.dma_start(out=t_bh[:], in_=bh[:])
            xhv00 = xhv[0, 0].rearrange(
                "p (hl gg k w) -> p hl gg k w", hl=2, gg=GPG, k=8)
            xt00a = xtp.tile([128, 2, 1, 8, 64], f16, tag="xt0a",
                             name="xt00a", bufs=1)
            nc.sync.dma_start(out=xt00a[:], in_=xhv00[:, :, 0:1, :, :])
            xt00b = xtp.tile([128, 2, GPG - 1, 8, 64], f16, tag="xt0b",
                             name="xt00b", bufs=1)
            nc.scalar.dma_start(out=xt00b[:], in_=xhv00[:, :, 1:GPG, :, :])
            if n_terms == 3:
                t_bl = cst.tile([128, 3 * 124], f16)
                nc.sync.dma_start(out=t_bl[:], in_=bl[:])
            # gate the big topk-table loads behind the first conv tile so
            # they don't compete for HBM bandwidth during startup
            gate = cst.tile([1, 64], f16)
            nc.gpsimd.dma_start(out=gate[:], in_=xt00a[0:1, 0, 0, 0, 0:64])
            t_eo = cst.tile([4, 512], f16)
            nc.gpsimd.dma_start(out=t_eo[:], in_=eo[:])
            t_mb1 = cst.tile([1, 128], f32)
            nc.gpsimd.dma_start(out=t_mb1[:], in_=mb1[:])
            t_ltm = cst.tile([128, 4 * 512], f32)
            nc.gpsimd.dma_start(out=t_ltm[:], in_=ltm[:])
            t_cvw = cst.tile([128, 512], f32)
            nc.gpsimd.dma_start(out=t_cvw[:], in_=cvw[:])
            rpi = cst.tile([128, 2], i32)
            nc.gpsimd.iota(rpi[:], pattern=[[128, 2]], base=0,
                           channel_multiplier=1)
            t_rpos = cst.tile([128, 2], f32)
            nc.vector.tensor_copy(t_rpos[:], rpi[:])
            ident = cst.tile([128, 128], f32)
            make_identity(nc, ident[:])
            ones = cst.tile([128, 512], f32)
            nc.vector.memset(ones[:], 1.0)
            t_mb = cst.tile([128, 1], f32)

            def emit_mb():
                """[1,128] -> [128,1] per-partition baseline via PE."""
                pm = ptp.tile([128, 128], f32, tag="tp")
                nc.tensor.transpose(pm[:, 0:1], t_mb1[:, :], ident[:1, :1])
                nc.vector.tensor_copy(t_mb[:], pm[:, 0:1])

            def emit_group(s, G, st, xt=None, xt2=None):
                """load + conv matmuls + |.| row-sums for tile G of sample s."""
                if xt is None:
                    xt = xtp.tile([128, 2, GPG, 8, 64], f16, tag="xt")
                    ldeng = nc.sync if (G % 2 == 0) else nc.scalar
                    ldeng.dma_start(out=xt[:], in_=xhv[s, G])
                # G0-3 row-sums land in Ra, G4-7 in Rb: separate tiles give
                # exact deps so half the score work runs mid-conv.
                Rh = st["Ra"] if G < 4 else st["Rb"]
                gbase = 0 if G < 4 else 16

                def rhs_of(hl, gg, lo, hi):
                    if xt2 is not None and gg > 0:
                        return xt2[:, hl, gg - 1, :, lo:hi]
                    return xt[:, hl, 0 if xt2 is not None else gg, :, lo:hi]
                for gp2 in range(GPG // 2):
                    pc = pcp.tile([124, 2, 512], f32, tag="pc")
                    for half in range(2):
                        gg = gp2 * 2 + half
                        mms = [(t_bh, 0), (t_bh, 1)]
                        if n_terms == 3:
                            mms.append((t_bl, 0))
                        nmm = 3 * len(mms)
                        im = 0
                        for dj in range(3):
                            for (tb, hl) in mms:
                                nc.tensor.matmul(
                                    pc[:, half, 0:496],
                                    lhsT=tb[:, dj * 124:(dj + 1) * 124],
                                    rhs=rhs_of(hl, gg, dj, dj + 62),
                                    start=(im == 0), stop=(im == nmm - 1))
                                im += 1
                    g0 = G * GPG + gp2 * 2 - gbase
                    nc.vector.tensor_reduce(
                        out=Rh[:, g0 * 8:(g0 + 2) * 8].rearrange(
                            "p (b k) -> p b k", b=2),
                        in_=pc[:, :, 0:496].rearrange(
                            "p b (k w) -> p b k w", k=8),
                        axis=Ax.X, op=Alu.add, apply_absolute_value=True)

            def emit_scores_fc(st, fc):
                """half of the per-channel score reduction (fc0 can run
                mid-conv of its own sample; fc1 right after the last tile)."""
                Rh = st["Ra"] if fc == 0 else st["Rb"]
                Rp = rp.tile([124, 128], f32, tag=f"Rp{fc}", name=f"Rp{fc}")
                nc.vector.tensor_scalar(
                    Rp[:], Rh[:], t_mb[:124, :1], None, op0=Alu.subtract)
                if fc == 0:
                    st["sc"] = sp.tile([128, 4], f32, tag="sc", name="sc")
                sc = st["sc"]
                ptr = ptp.tile([128, 128], f32, tag="tp")
                nc.tensor.transpose(
                    ptr[:, :124], Rp[:, :], ident[:124, :124])
                nc.vector.tensor_reduce(
                    out=sc[:, fc * 2:fc * 2 + 2],
                    in_=ptr[:, :124].rearrange("p (par i) -> p par i", par=2),
                    axis=Ax.X, op=Alu.add)

            def emit_split(st):
                """exact fp16 hi/lo split of sc + the bit-consistent scalar
                sstar = fp32(hi) + fp32(lo) (same arithmetic as PSUM)."""
                sc = st["sc"]
                sch16 = sp.tile([128, 4], f16, tag="sch16")
                nc.vector.tensor_copy(sch16[:], sc[:])
                schf = sp.tile([128, 4], f32, tag="schf")
                nc.vector.tensor_copy(schf[:], sch16[:])
                dif = sp.tile([128, 4], f32, tag="dif")
                with nc.allow_low_precision(reason="exact residual"):
                    nc.vector.tensor_tensor(
                        out=dif[:], in0=sc[:], in1=schf[:], op=Alu.subtract)
                scl16 = sp.tile([128, 4], f16, tag="scl16")
                nc.vector.tensor_copy(scl16[:], dif[:])
                sclf = sp.tile([128, 4], f32, tag="sclf")
                nc.vector.tensor_copy(sclf[:], scl16[:])
                sstar = sp.tile([128, 4], f32, tag="sstar")
                with nc.allow_low_precision(reason="matches psum add"):
                    nc.vector.tensor_tensor(
                        out=sstar[:], in0=schf[:], in1=sclf[:], op=Alu.add)
                st["schf"], st["sclf"], st["sstar"] = schf, sclf, sstar

            def tr_row16(src, tag):
                """[128,4] f32 -> [4,128] f16 row tile via PE transpose."""
                tp4 = ptp.tile([4, 128], f32, tag="tp4")
                nc.tensor.transpose(tp4[:], src[:, 0:4], ident[:128, :128])
                r16 = sp.tile([4, 128], f16, tag=tag, name="r16_" + tag)
                nc.vector.tensor_copy(r16[:], tp4[:])
                return r16

            def emit_bcast_scores(st):
                """fp16 hi/lo selector matmuls -> sbz [128,512] psum."""
                s = st["s"]
                slh = tr_row16(st["schf"], "slh")
                sll = tr_row16(st["sclf"], "sll")
                sbz = pbz.tile([128, 512], f32, tag="bz", name="sbz")
                for q in range(4):
                    nc.tensor.matmul(
                        sbz[:, q * 128:(q + 1) * 128],
                        lhsT=t_eo[:, q * 128:(q + 1) * 128],
                        rhs=slh[:, :], start=True, stop=False)
                    nc.tensor.matmul(
                        sbz[:, q * 128:(q + 1) * 128],
                        lhsT=t_eo[:, q * 128:(q + 1) * 128],
                        rhs=sll[:, :], start=False, stop=True)
                nc.gpsimd.dma_start(out=sdbg[s * 8:s * 8 + 4, :], in_=slh[:])
                nc.gpsimd.dma_start(out=sdbg[s * 8 + 4:s * 8 + 8, :],
                                    in_=sll[:])
                st["sbz"] = sbz

            def emit_ranks_g(st):
                """greater-counts (DVE; gpsimd lacks the TensorScalarPtr op)."""
                sstar, sbz = st["sstar"], st["sbz"]
                cntg = sp.tile([128, 4], f32, tag="cntg")
                for q in range(4):
                    junk = sp.tile([128, 512], f32, tag="junkg")
                    nc.vector.scalar_tensor_tensor(
                        out=junk[:], in0=sbz[:], scalar=sstar[:, q:q + 1],
                        in1=ones[:], op0=Alu.is_gt, op1=Alu.mult,
                        accum_out=cntg[:, q:q + 1])
                st["cntg"] = cntg

            def emit_ranks_e(st):
                """tie-break counts on DVE + combine -> ranks [128,4]."""
                sstar, sbz = st["sstar"], st["sbz"]
                cnte = sp.tile([128, 4], f32, tag="cnte")
                for q in range(4):
                    junk = sp.tile([128, 512], f32, tag="junk")
                    nc.vector.scalar_tensor_tensor(
                        out=junk[:], in0=sbz[:], scalar=sstar[:, q:q + 1],
                        in1=t_ltm[:, q * 512:(q + 1) * 512],
                        op0=Alu.is_equal, op1=Alu.mult,
                        accum_out=cnte[:, q:q + 1])
                ranks = sp.tile([128, 4], f32, tag="ranks")
                with nc.allow_low_precision(reason="exact small-int add"):
                    nc.vector.tensor_tensor(
                        out=ranks[:], in0=st["cntg"][:], in1=cnte[:],
                        op=Alu.add)
                st["ranks"] = ranks

            def emit_bcast_ranks(st):
                """ranks are exact ints <=511: single fp16 selector pass."""
                rkh = tr_row16(st["ranks"], "rkh")
                rb = pbz.tile([128, 512], f32, tag="bz", name="rb")
                for q in range(4):
                    nc.tensor.matmul(
                        rb[:, q * 128:(q + 1) * 128],
                        lhsT=t_eo[:, q * 128:(q + 1) * 128],
                        rhs=rkh[:, :], start=True, stop=True)
                st["rb"] = rb

            def emit_invert(st):
                """inverse permutation: eraw[p,rc] = xh row with rank rc*128+p."""
                s, rb = st["s"], st["rb"]
                invf = sp.tile([128, 2], f32, tag="invf")
                for rc in range(2):
                    junk = sp.tile([128, 512], f32, tag="junk")
                    nc.vector.scalar_tensor_tensor(
                        out=junk[:], in0=rb[:], scalar=t_rpos[:, rc:rc + 1],
                        in1=t_cvw[:], op0=Alu.is_equal, op1=Alu.mult,
                        accum_out=invf[:, rc:rc + 1])
                erf = sp.tile([128, 2], f32, tag="erf")
                nc.vector.tensor_scalar(
                    erf[:], invf[:], float(s * C), None, op0=Alu.add)
                eraw = sp.tile([128, 2], i32, tag="eraw")
                nc.vector.tensor_copy(eraw[:], erf[:])
                st["eraw"] = eraw

            def emit_gather(st, rc):
                """gather one half of the selected fp16 planes into SBUF."""
                if rc == 0:
                    st["gt"] = gp.tile([128, 2, PLANE], f16, tag="gt",
                                       name="gt")
                nc.gpsimd.indirect_dma_start(
                    out=st["gt"][:, rc, :], out_offset=None, in_=xh[:],
                    in_offset=bass.IndirectOffsetOnAxis(
                        ap=st["eraw"][:, rc:rc + 1], axis=0))

            def emit_write(st, rc):
                """write one half of the output, casting fp16->fp32."""
                s = st["s"]
                nc.gpsimd.dma_start(
                    out=out[s * K + rc * 128:s * K + (rc + 1) * 128, :],
                    in_=st["gt"][:, rc, :])

            # software pipeline: topk(s-1) interleaved into conv stream (s);
            # the fc0 half-score of the CURRENT sample runs at its own G6.
            HOOKS = {0: lambda st: emit_scores_fc(st, 1), 1: emit_split,
                     2: emit_bcast_scores, 3: emit_ranks_g,
                     4: emit_ranks_e, 5: emit_bcast_ranks,
                     6: lambda st: (emit_invert(st), emit_gather(st, 0),
                                    emit_gather(st, 1)),
                     7: lambda st: (emit_write(st, 0), emit_write(st, 1))}
            prev = None
            for s in range(SPC):
                st = {"s": s}
                st["Ra"] = rp.tile([124, 128], f32, tag="Ra", name="Ra")
                st["Rb"] = rp.tile([124, 128], f32, tag="Rb", name="Rb")
                for G in range(NGT):
                    if prev is not None and G in HOOKS:
                        HOOKS[G](prev)
                    if s == 0 and G == 5:
                        emit_mb()
                    if s == 0 and G == 0:
                        emit_group(s, G, st, xt=xt00a, xt2=xt00b)
                    else:
                        emit_group(s, G, st)
                    if G == 6:
                        emit_scores_fc(st, 0)
                prev = st
            # drain the last sample (fc0 already emitted at its G6)
            emit_scores_fc(prev, 1)
            emit_split(prev)
            emit_bcast_scores(prev)
            emit_ranks_g(prev)
            emit_ranks_e(prev)
            emit_bcast_ranks(prev)
            emit_invert(prev)
            emit_gather(prev, 0)
            emit_gather(prev, 1)
            emit_write(prev, 0)
            emit_write(prev, 1)
    nc.compile()
    _nc_cache[n_terms] = nc
    return nc


def _host_inputs(x: np.ndarray, weight: np.ndarray):
    w = weight.reshape(3, 3).astype(np.float32)
    wh = w.astype(np.float16)
    exact16 = bool(np.all(wh.astype(np.float32) == w))
    n_terms = 2 if exact16 else 3

    def banded(wcol):
        Bm = np.zeros((128, 3 * 124), dtype=np.float64)
        for dj in range(3):
            for half in range(2):
                for i in range(HO):
                    for t in range(3):
                        Bm[half * 64 + i + t, dj * 124 + half * 62 + i] = wcol[t, dj]
        return Bm

    Bfull = banded(w.astype(np.float64))
    bh_np = Bfull.astype(np.float16)
    bl_np = (Bfull - bh_np.astype(np.float64)).astype(np.float16)

    # baseline m: mean |conv| row-sum from one plane (ordering-neutral shift)
    p0 = x[0, 0].astype(np.float32)
    c0 = np.zeros((HO, WO), dtype=np.float32)
    for di in range(3):
        for dj in range(3):
            c0 += w[di, dj] * p0[di:di + HO, dj:dj + WO]
    m = np.float32(round(float(np.abs(c0).sum(axis=1).mean())))
    mb1_np = np.full((1, 128), m, dtype=np.float32)

    # free position j = q*128 + f <-> channel chanmap[j]
    p = np.arange(128)
    j = np.arange(512)
    q_of_j, f_of_j = j // 128, j % 128
    chanmap = 256 * (q_of_j // 2) + 2 * f_of_j + (q_of_j % 2)
    # ltm[p, q*512+j] = 1 if chan(j) < chan(p, q)  (tie-break mask)
    ltm_np = np.zeros((128, 4 * 512), dtype=np.float32)
    for q in range(4):
        chan_pq = 256 * (q // 2) + 2 * p + (q % 2)
        ltm_np[:, q * 512:(q + 1) * 512] = (
            chanmap[None, :] < chan_pq[:, None]).astype(np.float32)
    cvw_np = np.broadcast_to(
        chanmap.astype(np.float32)[None, :], (128, 512)).copy()
    eo_np = np.zeros((4, 512), dtype=np.float16)
    for q in range(4):
        eo_np[q, q * 128:(q + 1) * 128] = 1.0
    shared = dict(bh=bh_np, mb1=mb1_np, ltm=ltm_np, cvw=cvw_np, eo=eo_np)
    if n_terms == 3:
        shared["bl"] = bl_np
    return n_terms, shared


def _split_pair_layout(xc: np.ndarray) -> np.ndarray:
    """fp16 (hi, lo) split of one core's x in the conv pair-layout.

    xc: [SPC*C, PLANE] fp32 ->
    [SPC*NGT*128, 2*GPG*8*64] fp16 where row (s, G, par*64+h) holds
    [hl, gg, k, w] contiguously (4KB per DMA descriptor).
    """
    xh = xc.astype(np.float16)
    xl = (xc - xh.astype(np.float32)).astype(np.float16)
    # channel c = ((G*GPG + gg)*8 + k)*2 + par
    # [2hl, s, G, gg, k, par, h, w]
    arr = np.stack([xh, xl]).reshape(2, SPC, NGT, GPG, 8, 2, H, W)
    # -> [s, G, par, h, hl, gg, k, w]
    arr = arr.transpose(1, 2, 5, 6, 0, 3, 4, 7)
    return np.ascontiguousarray(arr).reshape(SPC * NGT * 128, 2 * GPG * 8 * 64)


def run(x, weight, trace=False):
    x = np.ascontiguousarray(np.asarray(x, dtype=np.float32))
    weight = np.asarray(weight, dtype=np.float32)
    assert x.shape == (B, C, H, W), x.shape
    n_terms, shared = _host_inputs(x, weight)
    nc = _build_nc(n_terms)
    in_maps = []
    for d in range(NCORES):
        im = dict(shared)
        xc = x[d * SPC:(d + 1) * SPC].reshape(SPC * C, PLANE)
        im["xh"] = xc.astype(np.float16)
        im["xhl"] = _split_pair_layout(xc)
        in_maps.append(im)
    res = run_bass_kernel_spmd(nc, in_maps, core_ids=list(range(NCORES)),
                               trace=trace)
    outs = [res.results[d]["out"].reshape(SPC, K, H, W) for d in range(NCORES)]
    return np.concatenate(outs, axis=0), res


def kernel(x, weight):
    out, _ = run(x, weight, trace=False)
    return out


# revision 26
# speedup vs baseline: 1.1867x; 1.1138x over previous
"""Trainium2 Bass kernel for nn_Curvature (topk_masking).

Pipeline per NeuronCore (8 cores, 4 samples each, pure data parallel):
  1. Host pre-splits x into an exact fp16 (hi, lo) pair and pre-shuffles it
     into the conv pair-layout [128p = 2ch x 64rows, (hi/lo, 4gg, 8k, 64w)]
     so every DMA descriptor is a 4KB contiguous run (full ring rate).
  2. Depthwise 3x3 conv as 6 accumulating PE matmuls per 16-channel group
     against banded stationary matrices built from the weight (one per
     column shift dj; hi and lo streamed through the same stationary).
  3. |conv| row-sums on DVE (tensor_reduce with absolute value, two groups
     per op), baseline-subtracted for fp32 accuracy, then per-channel
     totals via PE transpose + DVE reduce -> per-sample channel scores.
  4. Top-k (k=256) as counting-rank: rank(c) = #{j: s_j > s_c} +
     #{j < c: s_j == s_c} (matches jax.lax.top_k tie-breaking). The
     all-channel score row is broadcast to [128, 512] PSUM with a PE
     transpose + fp16 hi/lo selector matmuls; the comparison scalar is
     recomputed as fp32(hi)+fp32(lo) on DVE so it is bit-identical to the
     PSUM value (counting stays exact). Greater-counts run on GpSimd in
     parallel with tie-break counts on DVE. The inverse permutation is
     an is_equal reduction against the (exact-integer) fp16-broadcast
     ranks -- no DRAM scatter round trip.
  5. Gather the selected planes from a host-staged fp16 copy by rank via
     indirect DMA (8KB descriptors) and write the output with SWDGE
     fp16->fp32 casting DMAs. Gather+write live on the gpsimd queue so
     the sync/scalar HWDGE rings only ever stream conv tiles.
  Scores/topk for sample s-1 are emitted interleaved into sample s's conv
  stream; only the last sample's topk+gather is exposed as a short tail.
"""
import sys
import numpy as np

sys.path.insert(0, "/opt/trn_rl_repo")

import concourse.bacc as bacc
import concourse.bass as bass
import concourse.mybir as mybir
from concourse.masks import make_identity
from concourse.tile import TileContext
from concourse.bass_utils import run_bass_kernel_spmd

B, C, H, W = 32, 512, 64, 64
K = C // 2                 # 256 channels kept
NCORES = 8
SPC = B // NCORES          # samples per core = 4
HO, WO = H - 2, W - 2      # 62 x 62 valid conv output
NG = C // 16               # 32 groups of 16 channels (8 pairs)
PLANE = H * W
GPG = 4                    # groups per DMA load
NGT = NG // GPG            # 8 tile-loads per sample

_nc_cache = {}


def _build_nc(n_terms: int):
    """One SPMD program: SPC samples, full score+topk+gather pipeline."""
    if n_terms in _nc_cache:
        return _nc_cache[n_terms]
    nc = bacc.Bacc()
    dt = mybir.dt
    f32, f16, i32 = dt.float32, dt.float16, dt.int32
    Alu = mybir.AluOpType
    Ax = mybir.AxisListType

    xh = nc.declare_dram_parameter("xh", [SPC * C, PLANE], f16, isOutput=False)
    xhl = nc.declare_dram_parameter(
        "xhl", [SPC * NGT * 128, 2 * GPG * 8 * 64], f16, isOutput=False)
    bh = nc.declare_dram_parameter("bh", [128, 3 * 124], f16, isOutput=False)
    if n_terms == 3:
        bl = nc.declare_dram_parameter("bl", [128, 3 * 124], f16, isOutput=False)
    mb1 = nc.declare_dram_parameter("mb1", [1, 128], f32, isOutput=False)
    ltm = nc.declare_dram_parameter("ltm", [128, 4 * 512], f32, isOutput=False)
    cvw = nc.declare_dram_parameter("cvw", [128, 512], f32, isOutput=False)
    eo = nc.declare_dram_parameter("eo", [4, 512], f16, isOutput=False)
    out = nc.declare_dram_parameter("out", [SPC * K, PLANE], f32, isOutput=True)
    sdbg = nc.declare_dram_parameter("sdbg", [SPC * 8, 128], f32, isOutput=True)

    xhv = xhl[:].rearrange("(s G p) f -> s G p f", s=SPC, G=NGT, p=128)

    with TileContext(nc) as tc:
        with tc.tile_pool(name="cst", bufs=1) as cst, \
             tc.tile_pool(name="xtp", bufs=10) as xtp, \
             tc.tile_pool(name="rp", bufs=2) as rp, \
             tc.tile_pool(name="sp", bufs=2) as sp, \
             tc.tile_pool(name="gp", bufs=2) as gp, \
             tc.tile_pool(name="pcp", bufs=2, space="PSUM") as pcp, \
             tc.tile_pool(name="ptp", bufs=1, space="PSUM") as ptp, \
             tc.tile_pool(name="pbz", bufs=1, space="PSUM") as pbz:

            # critical path first: bh then the first conv tile on the sync
            # ring; everything tiny or big-but-late goes on gpsimd (SWDGE)
            # so the HWDGE rings never see small-descriptor storms.
            t_bh = cst.tile([128, 3 * 124], f16)
            nc.sync.dma_start(out=t_bh[:], in_=bh[:])
            xhv00 = xhv[0, 0].rearrange(
                "p (hl gg k w) -> p hl gg k w", hl=2, gg=GPG, k=8)
            xt00a = xtp.tile([128, 2, 1, 8, 64], f16, tag="xt0a",
                             name="xt00a", bufs=1)
            nc.sync.dma_start(out=xt00a[:], in_=xhv00[:, :, 0:1, :, :])
            xt00b = xtp.tile([128, 2, GPG - 1, 8, 64], f16, tag="xt0b",
                             name="xt00b", bufs=1)
            nc.scalar.dma_start(out=xt00b[:], in_=xhv00[:, :, 1:GPG, :, :])
            if n_terms == 3:
                t_bl = cst.tile([128, 3 * 124], f16)
                nc.sync.dma_start(out=t_bl[:], in_=bl[:])
            # gate the big topk-table loads behind the first conv tile so
            # they don't compete for HBM bandwidth during startup
            gate = cst.tile([1, 64], f16)
            nc.gpsimd.dma_start(out=gate[:], in_=xt00a[0:1, 0, 0, 0, 0:64])
            t_eo = cst.tile([4, 512], f16)
            nc.gpsimd.dma_start(out=t_eo[:], in_=eo[:])
            t_mb1 = cst.tile([1, 128], f32)
            nc.gpsimd.dma_start(out=t_mb1[:], in_=mb1[:])
            t_ltm = cst.tile([128, 4 * 512], f32)
            nc.gpsimd.dma_start(out=t_ltm[:], in_=ltm[:])
            t_cvw = cst.tile([128, 512], f32)
            nc.gpsimd.dma_start(out=t_cvw[:], in_=cvw[:])
            rpi = cst.tile([128, 2], i32)
            nc.gpsimd.iota(rpi[:], pattern=[[128, 2]], base=0,
                           channel_multiplier=1)
            t_rpos = cst.tile([128, 2], f32)
            nc.vector.tensor_copy(t_rpos[:], rpi[:])
            ident = cst.tile([128, 128], f32)
            make_identity(nc, ident[:])
            ones = cst.tile([128, 512], f32)
            nc.vector.memset(ones[:], 1.0)
            t_mb = cst.tile([128, 1], f32)

            def emit_mb():
                """[1,128] -> [128,1] per-partition baseline via PE."""
                pm = ptp.tile([128, 128], f32, tag="tp")
                nc.tensor.transpose(pm[:, 0:1], t_mb1[:, :], ident[:1, :1])
                nc.vector.tensor_copy(t_mb[:], pm[:, 0:1])

            def emit_group(s, G, st, xt=None, xt2=None):
                """load + conv matmuls + |.| row-sums for tile G of sample s."""
                if xt is None:
                    xt = xtp.tile([128, 2, GPG, 8, 64], f16, tag="xt")
                    ldeng = nc.sync if (G % 2 == 0) else nc.scalar
                    ldeng.dma_start(out=xt[:], in_=xhv[s, G])
                # G0-3 row-sums land in Ra, G4-7 in Rb: separate tiles give
                # exact deps so half the score work runs mid-conv.
                Rh = st["Ra"] if G < 4 else st["Rb"]
                gbase = 0 if G < 4 else 16

                def rhs_of(hl, gg, lo, hi):
                    if xt2 is not None and gg > 0:
                        return xt2[:, hl, gg - 1, :, lo:hi]
                    return xt[:, hl, 0 if xt2 is not None else gg, :, lo:hi]
                for gp2 in range(GPG // 2):
                    pc = pcp.tile([124, 2, 512], f32, tag="pc")
                    for half in range(2):
                        gg = gp2 * 2 + half
                        mms = [(t_bh, 0), (t_bh, 1)]
                        if n_terms == 3:
                            mms.append((t_bl, 0))
                        nmm = 3 * len(mms)
                        im = 0
                        for dj in range(3):
                            for (tb, hl) in mms:
                                nc.tensor.matmul(
                                    pc[:, half, 0:496],
                                    lhsT=tb[:, dj * 124:(dj + 1) * 124],
                                    rhs=rhs_of(hl, gg, dj, dj + 62),
                                    start=(im == 0), stop=(im == nmm - 1))
                                im += 1
                    g0 = G * GPG + gp2 * 2 - gbase
                    nc.vector.tensor_reduce(
                        out=Rh[:, g0 * 8:(g0 + 2) * 8].rearrange(
                            "p (b k) -> p b k", b=2),
                        in_=pc[:, :, 0:496].rearrange(
                            "p b (k w) -> p b k w", k=8),
                        axis=Ax.X, op=Alu.add, apply_absolute_value=True)

            def emit_scores_fc(st, fc):
                """half of the per-channel score reduction (fc0 can run
                mid-conv of its own sample; fc1 right after the last tile)."""
                Rh = st["Ra"] if fc == 0 else st["Rb"]
                Rp = rp.tile([124, 128], f32, tag=f"Rp{fc}", name=f"Rp{fc}")
                nc.vector.tensor_scalar(
                    Rp[:], Rh[:], t_mb[:124, :1], None, op0=Alu.subtract)
                if fc == 0:
                    st["sc"] = sp.tile([128, 4], f32, tag="sc", name="sc")
                sc = st["sc"]
                ptr = ptp.tile([128, 128], f32, tag="tp")
                nc.tensor.transpose(
                    ptr[:, :124], Rp[:, :], ident[:124, :124])
                nc.vector.tensor_reduce(
                    out=sc[:, fc * 2:fc * 2 + 2],
                    in_=ptr[:, :124].rearrange("p (par i) -> p par i", par=2),
                    axis=Ax.X, op=Alu.add)

            def emit_split(st):
                """exact fp16 hi/lo split of sc + the bit-consistent scalar
                sstar = fp32(hi) + fp32(lo) (same arithmetic as PSUM)."""
                sc = st["sc"]
                sch16 = sp.tile([128, 4], f16, tag="sch16")
                nc.vector.tensor_copy(sch16[:], sc[:])
                schf = sp.tile([128, 4], f32, tag="schf")
                nc.vector.tensor_copy(schf[:], sch16[:])
                dif = sp.tile([128, 4], f32, tag="dif")
                with nc.allow_low_precision(reason="exact residual"):
                    nc.vector.tensor_tensor(
                        out=dif[:], in0=sc[:], in1=schf[:], op=Alu.subtract)
                scl16 = sp.tile([128, 4], f16, tag="scl16")
                nc.vector.tensor_copy(scl16[:], dif[:])
                sclf = sp.tile([128, 4], f32, tag="sclf")
                nc.vector.tensor_copy(sclf[:], scl16[:])
                sstar = sp.tile([128, 4], f32, tag="sstar")
                with nc.allow_low_precision(reason="matches psum add"):
                    nc.vector.tensor_tensor(
                        out=sstar[:], in0=schf[:], in1=sclf[:], op=Alu.add)
                st["schf"], st["sclf"], st["sstar"] = schf, sclf, sstar

            def tr_row16(src, tag):
                """[128,4] f32 -> [4,128] f16 row tile via PE transpose."""
                tp4 = ptp.tile([4, 128], f32, tag="tp4")
                nc.tensor.transpose(tp4[:], src[:, 0:4], ident[:128, :128])
                r16 = sp.tile([4, 128], f16, tag=tag, name="r16_" + tag)
                nc.vector.tensor_copy(r16[:], tp4[:])
                return r16

            def emit_bcast_scores(st):
                """fp16 hi/lo selector matmuls -> sbz [128,512] psum."""
                s = st["s"]
                slh = tr_row16(st["schf"], "slh")
                sll = tr_row16(st["sclf"], "sll")
                sbz = pbz.tile([128, 512], f32, tag="bz", name="sbz")
                for q in range(4):
                    nc.tensor.matmul(
                        sbz[:, q * 128:(q + 1) * 128],
                        lhsT=t_eo[:, q * 128:(q + 1) * 128],
                        rhs=slh[:, :], start=True, stop=False)
                    nc.tensor.matmul(
                        sbz[:, q * 128:(q + 1) * 128],
                        lhsT=t_eo[:, q * 128:(q + 1) * 128],
                        rhs=sll[:, :], start=False, stop=True)
                nc.gpsimd.dma_start(out=sdbg[s * 8:s * 8 + 4, :], in_=slh[:])
                nc.gpsimd.dma_start(out=sdbg[s * 8 + 4:s * 8 + 8, :],
                                    in_=sll[:])
                st["sbz"] = sbz

            def emit_ranks_g(st):
                """greater-counts (DVE; gpsimd lacks the TensorScalarPtr op)."""
                sstar, sbz = st["sstar"], st["sbz"]
                cntg = sp.tile([128, 4], f32, tag="cntg")
                for q in range(4):
                    junk = sp.tile([128, 512], f32, tag="junkg")
                    nc.vector.scalar_tensor_tensor(
                        out=junk[:], in0=sbz[:], scalar=sstar[:, q:q + 1],
                        in1=ones[:], op0=Alu.is_gt, op1=Alu.mult,
                        accum_out=cntg[:, q:q + 1])
                st["cntg"] = cntg

            def emit_ranks_e(st):
                """tie-break counts on DVE + combine -> ranks [128,4]."""
                sstar, sbz = st["sstar"], st["sbz"]
                cnte = sp.tile([128, 4], f32, tag="cnte")
                for q in range(4):
                    junk = sp.tile([128, 512], f32, tag="junk")
                    nc.vector.scalar_tensor_tensor(
                        out=junk[:], in0=sbz[:], scalar=sstar[:, q:q + 1],
                        in1=t_ltm[:, q * 512:(q + 1) * 512],
                        op0=Alu.is_equal, op1=Alu.mult,
                        accum_out=cnte[:, q:q + 1])
                ranks = sp.tile([128, 4], f32, tag="ranks")
                with nc.allow_low_precision(reason="exact small-int add"):
                    nc.vector.tensor_tensor(
                        out=ranks[:], in0=st["cntg"][:], in1=cnte[:],
                        op=Alu.add)
                st["ranks"] = ranks

            def emit_bcast_ranks(st):
                """ranks are exact ints <=511: single fp16 selector pass."""
                rkh = tr_row16(st["ranks"], "rkh")
                rb = pbz.tile([128, 512], f32, tag="bz", name="rb")
                for q in range(4):
                    nc.tensor.matmul(
                        rb[:, q * 128:(q + 1) * 128],
                        lhsT=t_eo[:, q * 128:(q + 1) * 128],
                        rhs=rkh[:, :], start=True, stop=True)
                st["rb"] = rb

            def emit_invert(st):
                """inverse permutation: eraw[p,rc] = xh row with rank rc*128+p."""
                s, rb = st["s"], st["rb"]
                invf = sp.tile([128, 2], f32, tag="invf")
                for rc in range(2):
                    junk = sp.tile([128, 512], f32, tag="junk")
                    nc.vector.scalar_tensor_tensor(
                        out=junk[:], in0=rb[:], scalar=t_rpos[:, rc:rc + 1],
                        in1=t_cvw[:], op0=Alu.is_equal, op1=Alu.mult,
                        accum_out=invf[:, rc:rc + 1])
                erf = sp.tile([128, 2], f32, tag="erf")
                nc.vector.tensor_scalar(
                    erf[:], invf[:], float(s * C), None, op0=Alu.add)
                eraw = sp.tile([128, 2], i32, tag="eraw")
                nc.vector.tensor_copy(eraw[:], erf[:])
                st["eraw"] = eraw

            def emit_gather(st, rc):
                """gather one half of the selected fp16 planes into SBUF."""
                if rc == 0:
                    st["gt"] = gp.tile([128, 2, PLANE], f16, tag="gt",
                                       name="gt")
                nc.gpsimd.indirect_dma_start(
                    out=st["gt"][:, rc, :], out_offset=None, in_=xh[:],
                    in_offset=bass.IndirectOffsetOnAxis(
                        ap=st["eraw"][:, rc:rc + 1], axis=0))

            def emit_write(st, rc):
                """write one half of the output, casting fp16->fp32."""
                s = st["s"]
                nc.gpsimd.dma_start(
                    out=out[s * K + rc * 128:s * K + (rc + 1) * 128, :],
                    in_=st["gt"][:, rc, :])

            # software pipeline: topk(s-1) interleaved into conv stream (s);
            # the fc0 half-score of the CURRENT sample runs at its own G6.
            HOOKS = {0: lambda st: emit_scores_fc(st, 1), 1: emit_split,
                     2: emit_bcast_scores, 3: emit_ranks_g,
                     4: emit_ranks_e, 5: emit_bcast_ranks,
                     6: lambda st: (emit_invert(st), emit_gather(st, 0),
                                    emit_gather(st, 1)),
                     7: lambda st: (emit_write(st, 0), emit_write(st, 1))}
            prev = None
            for s in range(SPC):
                st = {"s": s}
                st["Ra"] = rp.tile([124, 128], f32, tag="Ra", name="Ra")
                st["Rb"] = rp.tile([124, 128], f32, tag="Rb", name="Rb")
                for G in range(NGT):
                    if prev is not None and G in HOOKS:
                        HOOKS[G](prev)
                    if s == 0 and G == 5:
                        emit_mb()
                    if s == 0 and G == 0:
                        emit_group(s, G, st, xt=xt00a, xt2=xt00b)
                    else:
                        emit_group(s, G, st)
                    if G == 6:
                        emit_scores_fc(st, 0)
                prev = st
            # drain the last sample (fc0 already emitted at its G6)
            emit_scores_fc(prev, 1)
            emit_split(prev)
            emit_bcast_scores(prev)
            emit_ranks_g(prev)
            emit_ranks_e(prev)
            emit_bcast_ranks(prev)
            emit_invert(prev)
            emit_gather(prev, 0)
            emit_gather(prev, 1)
            emit_write(prev, 0)
            emit_write(prev, 1)
    nc.compile()
    _nc_cache[n_terms] = nc
    return nc


def _host_inputs(x: np.ndarray, weight: np.ndarray):
    w = weight.reshape(3, 3).astype(np.float32)
    wh = w.astype(np.float16)
    exact16 = bool(np.all(wh.astype(np.float32) == w))
    n_terms = 2 if exact16 else 3

    def banded(wcol):
        Bm = np.zeros((128, 3 * 124), dtype=np.float64)
        for dj in range(3):
            for half in range(2):
                for i in range(HO):
                    for t in range(3):
                        Bm[half * 64 + i + t, dj * 124 + half * 62 + i] = wcol[t, dj]
        return Bm

    Bfull = banded(w.astype(np.float64))
    bh_np = Bfull.astype(np.float16)
    bl_np = (Bfull - bh_np.astype(np.float64)).astype(np.float16)

    # baseline m: mean |conv| row-sum from one plane (ordering-neutral shift)
    p0 = x[0, 0].astype(np.float32)
    c0 = np.zeros((HO, WO), dtype=np.float32)
    for di in range(3):
        for dj in range(3):
            c0 += w[di, dj] * p0[di:di + HO, dj:dj + WO]
    m = np.float32(round(float(np.abs(c0).sum(axis=1).mean())))
    mb1_np = np.full((1, 128), m, dtype=np.float32)

    # free position j = q*128 + f <-> channel chanmap[j]
    p = np.arange(128)
    j = np.arange(512)
    q_of_j, f_of_j = j // 128, j % 128
    chanmap = 256 * (q_of_j // 2) + 2 * f_of_j + (q_of_j % 2)
    # ltm[p, q*512+j] = 1 if chan(j) < chan(p, q)  (tie-break mask)
    ltm_np = np.zeros((128, 4 * 512), dtype=np.float32)
    for q in range(4):
        chan_pq = 256 * (q // 2) + 2 * p + (q % 2)
        ltm_np[:, q * 512:(q + 1) * 512] = (
            chanmap[None, :] < chan_pq[:, None]).astype(np.float32)
    cvw_np = np.broadcast_to(
        chanmap.astype(np.float32)[None, :], (128, 512)).copy()
    eo_np = np.zeros((4, 512), dtype=np.float16)
    for q in range(4):
        eo_np[q, q * 128:(q + 1) * 128] = 1.0
    shared = dict(bh=bh_np, mb1=mb1_np, ltm=ltm_np, cvw=cvw_np, eo=eo_np)
    if n_terms == 3:
        shared["bl"] = bl_np
    return n_terms, shared


def _split_pair_layout(xc: np.ndarray) -> np.ndarray:
    """fp16 (hi, lo) split of one core's x in the conv pair-layout.

    xc: [SPC*C, PLANE] fp32 ->
    [SPC*NGT*128, 2*GPG*8*64] fp16 where row (s, G, par*64+h) holds
    [hl, gg, k, w] contiguously (4KB per DMA descriptor).
    """
    xh = xc.astype(np.float16)
    xl = (xc - xh.astype(np.float32)).astype(np.float16)
    # channel c = ((G*GPG + gg)*8 + k)*2 + par
    # [2hl, s, G, gg, k, par, h, w]
    arr = np.stack([xh, xl]).reshape(2, SPC, NGT, GPG, 8, 2, H, W)
    # -> [s, G, par, h, hl, gg, k, w]
    arr = arr.transpose(1, 2, 5, 6, 0, 3, 4, 7)
    return np.ascontiguousarray(arr).reshape(SPC * NGT * 128, 2 * GPG * 8 * 64)


def run(x, weight, trace=False):
    x = np.ascontiguousarray(np.asarray(x, dtype=np.float32))
    weight = np.asarray(weight, dtype=np.float32)
    assert x.shape == (B, C, H, W), x.shape
    n_terms, shared = _host_inputs(x, weight)
    nc = _build_nc(n_terms)
    in_maps = []
    for d in range(NCORES):
        im = dict(shared)
        xc = x[d * SPC:(d + 1) * SPC].reshape(SPC * C, PLANE)
        im["xh"] = xc.astype(np.float16)
        im["xhl"] = _split_pair_layout(xc)
        in_maps.append(im)
    res = run_bass_kernel_spmd(nc, in_maps, core_ids=list(range(NCORES)),
                               trace=trace)
    outs = [res.results[d]["out"].reshape(SPC, K, H, W) for d in range(NCORES)]
    return np.concatenate(outs, axis=0), res


def kernel(x, weight):
    out, _ = run(x, weight, trace=False)
    return out
